# revision 6
# baseline (speedup 1.0000x reference)
"""Trainium2 Bass kernel for nn_ActorNetwork (GNN message passing actor).

Self-contained: hardcodes shapes B=32, K=64, D=4, DS=4, H=512, HH=256, NA=2.
Data-parallel over batch across 8 NeuronCores (4 samples/core), all params
replicated. Returns (mu, std) like the reference.

Host-side weight prep folds: LayerNorm affine into n1, the /counts
normalizations into the LN eps and the head-1 weights, n2 bias into the
head-1 biases. The edge-MLP first layer is split into U = A@o_i + b and
V = C@o_j so the [K,K,2D] edge input tensor is never materialized.
"""
import numpy as np

import concourse.bass as bass
import concourse.mybir as mybir
from concourse.bass_utils import run_bass_kernel_spmd
from concourse.tile import TileContext

# ---- problem constants ----
B, K, D, DS, H, HH, NA = 32, 64, 4, 4, 512, 256, 2
NCORES = 8
BSH = B // NCORES            # samples per core = 4
P = 128
FT = H // P                  # 4 feature tiles of hidden dim
TOK = BSH * K                # 256 node tokens per core
IBLK = 8                     # i-rows per edge chunk (8*64 = 512 tokens)
NCH = K // IBLK              # 8 chunks per sample
EPS_S = (K * K) * 1e-5       # LN eps scaled for un-normalized agg sums

F32 = mybir.dt.float32
F32R = mybir.dt.float32r
BF16 = mybir.dt.bfloat16
AF = mybir.ActivationFunctionType
ALU = mybir.AluOpType

# dtype knobs: 'f32' | 'f32r' | 'bf16' for the big edge matmuls,
# 'f32' | 'f32r' for the small matmuls elsewhere.
EDGE_DT = "f32"
SMALL_DT = "f32"


def _mm(ap, kind):
    if kind == "f32r" and ap.dtype == F32:
        return ap.bitcast(F32R)
    return ap


def _split_excess_waits(nc, max_waits=1):
    """walrus in this container rejects >~2 sem waits on one instruction
    (hits the Tile tail drain). Split excess waits onto same-engine NoOps."""
    for f in nc.m.functions:
        for bb in f.blocks:
            insts = list(bb.instructions)
            new_list = []
            changed = False
            for inst in insts:
                si = inst.sync_info
                if si is not None and si.on_wait and len(si.on_wait) > max_waits:
                    waits = list(si.on_wait)
                    extra, keep = waits[:-max_waits], waits[-max_waits:]
                    for k0 in range(0, len(extra), max_waits):
                        chunk = extra[k0 : k0 + max_waits]
                        nop = mybir.InstNoOp(
                            name=f"{inst.name}-wsplit-{k0}",
                            engine=inst.engine,
                            ins=[],
                            outs=[],
                            sync_info=mybir.SyncInfo(on_wait=chunk, on_update=[]),
                        )
                        new_list.append(nop)
                        changed = True
                    si.on_wait = keep
                new_list.append(inst)
            if changed:
                bb.instructions = new_list


def build_bass():
    edge_store = BF16 if EDGE_DT == "bf16" else F32
    nc = bass.Bass("TRN2", debug=False, num_devices=NCORES)

    def dp(nm, sh, dt=F32):
        return nc.declare_dram_parameter(nm, sh, dt, isOutput=False)

    obs_d = dp("obs", [BSH, D, K])
    st_d = dp("stateT", [DS, BSH])
    e1AT_d = dp("e1AT", [D, H])
    e1CT_d = dp("e1CT", [D, H])
    e2T_d = dp("e2T", [FT, P, H], edge_store)
    e3T_d = dp("e3T", [FT, P, H], edge_store)
    n1aT_d = dp("n1aT", [FT, P, H])
    n1oT_d = dp("n1oT", [D, H])
    n1sT_d = dp("n1sT", [DS, H])
    n2T_d = dp("n2T", [FT, P, HH])
    layerT_d = dp("layerT", [DS, H])
    mu1T_d = dp("mu1T", [2 * FT, P, 256])
    s1T_d = dp("s1T", [2 * FT, P, 256])
    mu2T_d = dp("mu2T", [2, P, 128])
    s2T_d = dp("s2T", [2, P, 128])
    mu3T_d = dp("mu3T", [P, NA])
    s3T_d = dp("s3T", [P, NA])
    bias_d = dp("bias_pack", [P, 32])
    mu_d = nc.declare_dram_parameter("mu", [NA, BSH], F32, isOutput=True)
    std_d = nc.declare_dram_parameter("std", [NA, BSH], F32, isOutput=True)

    with TileContext(nc) as tc:
        with (
            tc.tile_pool(name="w", bufs=1) as wp,
            tc.tile_pool(name="act", bufs=1) as pa,
            tc.tile_pool(name="chunk", bufs=2) as cp,
            tc.tile_pool(name="ps", bufs=8, space="PSUM") as pp,
        ):
            # ---------------- weight loads ----------------
            def wload(nm, dram, idx=None, dt=F32):
                src = dram[:] if idx is None else dram[idx]
                t = wp.tile(list(src.shape), dt, name=nm, tag=nm)
                nc.sync.dma_start(out=t, in_=src)
                return t

            e2w = [wload(f"e2w{k}", e2T_d, k, edge_store) for k in range(FT)]
            e3w = [wload(f"e3w{k}", e3T_d, k, edge_store) for k in range(FT)]
            e1Aw = wload("e1Aw", e1AT_d)
            e1Cw = wload("e1Cw", e1CT_d)
            n1aw = [wload(f"n1aw{k}", n1aT_d, k) for k in range(FT)]
            n1ow = wload("n1ow", n1oT_d)
            n1sw = wload("n1sw", n1sT_d)
            n2w = [wload(f"n2w{k}", n2T_d, k) for k in range(FT)]
            layerw = wload("layerw", layerT_d)
            mu1w = [wload(f"mu1w{k}", mu1T_d, k) for k in range(2 * FT)]
            s1w = [wload(f"s1w{k}", s1T_d, k) for k in range(2 * FT)]
            mu2w = [wload(f"mu2w{k}", mu2T_d, k) for k in range(2)]
            s2w = [wload(f"s2w{k}", s2T_d, k) for k in range(2)]
            mu3w = wload("mu3w", mu3T_d)
            s3w = wload("s3w", s3T_d)
            bias_t = wload("bias_t", bias_d)

            def bcol(i, rows=P):
                return bias_t[0:rows, i : i + 1]

            # ---------------- activations in ----------------
            o_all = pa.tile([D, TOK], F32, name="o_all", tag="o_all")
            nc.sync.dma_start(
                out=o_all[:].rearrange("d (b k) -> d b k", b=BSH),
                in_=obs_d[:].rearrange("b d k -> d b k"),
            )
            st_t = pa.tile([DS, BSH], F32, name="st_t", tag="st_t")
            nc.sync.dma_start(out=st_t, in_=st_d[:])
            state_bc = pa.tile([DS, TOK], F32, name="state_bc", tag="state_bc")
            nc.vector.tensor_copy(
                state_bc[:].rearrange("s (b k) -> s b k", b=BSH),
                st_t[:, :, None].broadcast_to([DS, BSH, K]),
            )

            # ---------------- U/V (edge layer 1, split) ----------------
            U_all, V_all, agg = [], [], []
            for m in range(FT):
                msl = slice(m * P, (m + 1) * P)
                pu = pp.tile([P, TOK], F32, name=f"pu{m}", tag="ps")
                nc.tensor.matmul(
                    pu, _mm(e1Aw[:, msl], SMALL_DT), _mm(o_all, SMALL_DT),
                    start=True, stop=True,
                )
                Um = pa.tile([P, TOK], F32, name=f"U{m}", tag=f"U{m}")
                nc.scalar.activation(Um, pu, AF.Identity, bias=bcol(0 + m))
                U_all.append(Um)

                pv = pp.tile([P, TOK], F32, name=f"pv{m}", tag="ps")
                nc.tensor.matmul(
                    pv, _mm(e1Cw[:, msl], SMALL_DT), _mm(o_all, SMALL_DT),
                    start=True, stop=True,
                )
                Vm = pa.tile([P, TOK], F32, name=f"V{m}", tag=f"V{m}")
                nc.vector.tensor_copy(Vm, pv)
                V_all.append(Vm)

                am = pa.tile([P, TOK], F32, name=f"agg{m}", tag=f"agg{m}")
                agg.append(am)

            # ---------------- edge MLP over K x K pairs ----------------
            for b in range(BSH):
                for ib in range(NCH):
                    i0 = b * K + ib * IBLK
                    h1 = []
                    for m in range(FT):
                        h1m = cp.tile([P, IBLK * K], edge_store,
                                      name=f"h1_{m}", tag=f"h1_{m}")
                        Ubc = U_all[m][:, i0 : i0 + IBLK][:, :, None].broadcast_to(
                            [P, IBLK, K])
                        Vbc = V_all[m][:, b * K : (b + 1) * K][:, None, :].broadcast_to(
                            [P, IBLK, K])
                        nc.vector.tensor_add(
                            h1m[:].rearrange("p (i j) -> p i j", i=IBLK), Ubc, Vbc)
                        nc.gpsimd.tensor_relu(h1m, h1m)
                        h1.append(h1m)
                    h2 = []
                    for m in range(FT):
                        msl = slice(m * P, (m + 1) * P)
                        ps2 = pp.tile([P, IBLK * K], F32, name=f"ps2_{m}", tag="ps")
                        for k2 in range(FT):
                            nc.tensor.matmul(
                                ps2, _mm(e2w[k2][:, msl], EDGE_DT),
                                _mm(h1[k2], EDGE_DT),
                                start=(k2 == 0), stop=(k2 == FT - 1),
                            )
                        h2m = cp.tile([P, IBLK * K], edge_store,
                                      name=f"h2_{m}", tag=f"h2_{m}")
                        nc.scalar.activation(h2m, ps2, AF.Relu, bias=bcol(4 + m))
                        h2.append(h2m)
                    for m in range(FT):
                        msl = slice(m * P, (m + 1) * P)
                        ps3 = pp.tile([P, IBLK * K], F32, name=f"ps3_{m}", tag="ps")
                        for k2 in range(FT):
                            nc.tensor.matmul(
                                ps3, _mm(e3w[k2][:, msl], EDGE_DT),
                                _mm(h2[k2], EDGE_DT),
                                start=(k2 == 0), stop=(k2 == FT - 1),
                            )
                        h3m = cp.tile([P, IBLK * K], edge_store,
                                      name=f"h3_{m}", tag=f"h3_{m}")
                        nc.scalar.activation(h3m, ps3, AF.Relu, bias=bcol(8 + m))
                        nc.vector.reduce_sum(
                            out=agg[m][:, i0 : i0 + IBLK],
                            in_=h3m[:].rearrange("p (i j) -> p i j", i=IBLK),
                            axis=mybir.AxisListType.X,
                        )

            # ---------------- LayerNorm over H (token stats via matmul) ----
            ones_col = pa.tile([P, 1], F32, name="ones_col", tag="ones_col")
            nc.vector.memset(ones_col, 1.0)
            ones_row = pa.tile([1, P], F32, name="ones_row", tag="ones_row")
            nc.vector.memset(ones_row, 1.0)

            sq = []
            for m in range(FT):
                sqm = pa.tile([P, TOK], F32, name=f"sq{m}", tag=f"sq{m}")
                nc.vector.tensor_mul(sqm, agg[m], agg[m])
                sq.append(sqm)
            ps_sum = pp.tile([1, TOK], F32, name="ps_sum", tag="ps")
            ps_ssq = pp.tile([1, TOK], F32, name="ps_ssq", tag="ps")
            for m in range(FT):
                nc.tensor.matmul(ps_sum, _mm(ones_col, SMALL_DT),
                                 _mm(agg[m], SMALL_DT),
                                 start=(m == 0), stop=(m == FT - 1))
            for m in range(FT):
                nc.tensor.matmul(ps_ssq, _mm(ones_col, SMALL_DT),
                                 _mm(sq[m], SMALL_DT),
                                 start=(m == 0), stop=(m == FT - 1))
            mean_r = pa.tile([1, TOK], F32, name="mean_r", tag="mean_r")
            nc.vector.tensor_scalar_mul(mean_r, ps_sum, 1.0 / H)
            msq_r = pa.tile([1, TOK], F32, name="msq_r", tag="msq_r")
            nc.vector.tensor_mul(msq_r, mean_r, mean_r)
            var_r = pa.tile([1, TOK], F32, name="var_r", tag="var_r")
            nc.vector.scalar_tensor_tensor(
                var_r, ps_ssq, 1.0 / H, msq_r, op0=ALU.mult, op1=ALU.subtract)
            eps_t = pa.tile([1, 1], F32, name="eps_t", tag="eps_t")
            nc.vector.memset(eps_t, EPS_S)
            sd_r = pa.tile([1, TOK], F32, name="sd_r", tag="sd_r")
            nc.scalar.activation(sd_r, var_r, AF.Sqrt, bias=eps_t)
            rstd_r = pa.tile([1, TOK], F32, name="rstd_r", tag="rstd_r")
            nc.vector.reciprocal(rstd_r, sd_r)

            ps_mb = pp.tile([P, TOK], F32, name="ps_mb", tag="ps")
            nc.tensor.matmul(ps_mb, _mm(ones_row, SMALL_DT),
                             _mm(mean_r, SMALL_DT), start=True, stop=True)
            mean_bc = pa.tile([P, TOK], F32, name="mean_bc", tag="mean_bc")
            nc.scalar.copy(mean_bc, ps_mb)
            ps_rb = pp.tile([P, TOK], F32, name="ps_rb", tag="ps")
            nc.tensor.matmul(ps_rb, _mm(ones_row, SMALL_DT),
                             _mm(rstd_r, SMALL_DT), start=True, stop=True)
            rstd_bc = pa.tile([P, TOK], F32, name="rstd_bc", tag="rstd_bc")
            nc.scalar.copy(rstd_bc, ps_rb)

            aggn = []
            for m in range(FT):
                anm = pa.tile([P, TOK], F32, name=f"aggn{m}", tag=f"aggn{m}")
                nc.vector.tensor_sub(anm, agg[m], mean_bc)
                nc.vector.tensor_mul(anm, anm, rstd_bc)
                aggn.append(anm)

            # ---------------- node MLP ----------------
            hn1 = []
            for m in range(FT):
                msl = slice(m * P, (m + 1) * P)
                psn = pp.tile([P, TOK], F32, name=f"psn1_{m}", tag="ps")
                for k2 in range(FT):
                    nc.tensor.matmul(psn, _mm(n1aw[k2][:, msl], SMALL_DT),
                                     _mm(aggn[k2], SMALL_DT),
                                     start=(k2 == 0), stop=False)
                nc.tensor.matmul(psn, _mm(n1ow[:, msl], SMALL_DT),
                                 _mm(o_all, SMALL_DT), start=False, stop=False)
                nc.tensor.matmul(psn, _mm(n1sw[:, msl], SMALL_DT),
                                 _mm(state_bc, SMALL_DT), start=False, stop=True)
                hm = pa.tile([P, TOK], F32, name=f"hn1_{m}", tag=f"hn1_{m}")
                nc.scalar.activation(hm, psn, AF.Relu, bias=bcol(12 + m))
                hn1.append(hm)

            pool_sum, pool_max = [], []
            for m2 in range(HH // P):
                msl = slice(m2 * P, (m2 + 1) * P)
                psn2 = pp.tile([P, TOK], F32, name=f"psn2_{m2}", tag="ps")
                for k2 in range(FT):
                    nc.tensor.matmul(psn2, _mm(n2w[k2][:, msl], SMALL_DT),
                                     _mm(hn1[k2], SMALL_DT),
                                     start=(k2 == 0), stop=(k2 == FT - 1))
                sm = pa.tile([P, BSH], F32, name=f"pool_s{m2}", tag=f"pool_s{m2}")
                nc.vector.reduce_sum(
                    out=sm, in_=psn2[:].rearrange("p (b k) -> p b k", b=BSH),
                    axis=mybir.AxisListType.X)
                mm_ = pa.tile([P, BSH], F32, name=f"pool_m{m2}", tag=f"pool_m{m2}")
                nc.vector.reduce_max(
                    out=mm_, in_=psn2[:].rearrange("p (b k) -> p b k", b=BSH),
                    axis=mybir.AxisListType.X)
                pool_sum.append(sm)
                pool_max.append(mm_)

            # ---------------- heads ----------------
            xs = []
            for m in range(FT):
                msl = slice(m * P, (m + 1) * P)
                pst = pp.tile([P, BSH], F32, name=f"pst{m}", tag="ps")
                nc.tensor.matmul(pst, _mm(layerw[:, msl], SMALL_DT),
                                 _mm(st_t, SMALL_DT), start=True, stop=True)
                xm = pa.tile([P, BSH], F32, name=f"xst{m}", tag=f"xst{m}")
                nc.scalar.activation(xm, pst, AF.Relu, bias=bcol(16 + m))
                xs.append(xm)
            xs += pool_sum + pool_max  # x = [st_feat, mean_pool, max_pool]

            def head(w1, w2, w3, bc1, bc2, tag):
                hl1 = []
                for m in range(2):
                    msl = slice(m * P, (m + 1) * P)
                    ph = pp.tile([P, BSH], F32, name=f"p{tag}1_{m}", tag="ps")
                    for k2 in range(2 * FT):
                        nc.tensor.matmul(ph, _mm(w1[k2][:, msl], SMALL_DT),
                                         _mm(xs[k2], SMALL_DT),
                                         start=(k2 == 0), stop=(k2 == 2 * FT - 1))
                    hm = pa.tile([P, BSH], F32, name=f"h{tag}1_{m}",
                                 tag=f"h{tag}1_{m}")
                    nc.scalar.activation(hm, ph, AF.Relu, bias=bcol(bc1 + m))
                    hl1.append(hm)
                ph2 = pp.tile([P, BSH], F32, name=f"p{tag}2", tag="ps")
                for k2 in range(2):
                    nc.tensor.matmul(ph2, _mm(w2[k2], SMALL_DT),
                                     _mm(hl1[k2], SMALL_DT),
                                     start=(k2 == 0), stop=(k2 == 1))
                hm2 = pa.tile([P, BSH], F32, name=f"h{tag}2", tag=f"h{tag}2")
                nc.scalar.activation(hm2, ph2, AF.Relu, bias=bcol(bc2))
                ph3 = pp.tile([NA, BSH], F32, name=f"p{tag}3", tag="ps")
                nc.tensor.matmul(ph3, _mm(w3, SMALL_DT), _mm(hm2, SMALL_DT),
                                 start=True, stop=True)
                return ph3

            ph3_mu = head(mu1w, mu2w, mu3w, 20, 22, "mu")
            mu_sb = pa.tile([NA, BSH], F32, name="mu_sb", tag="mu_sb")
            nc.scalar.activation(mu_sb, ph3_mu, AF.Identity, bias=bcol(23, rows=NA))
            nc.sync.dma_start(out=mu_d[:], in_=mu_sb)

            # softplus isn't co-loadable with sqrt in the ACT func sets;
            # emit the pre-softplus logits, host applies softplus+clip.
            ph3_s = head(s1w, s2w, s3w, 24, 26, "s")
            std_sb = pa.tile([NA, BSH], F32, name="std_sb", tag="std_sb")
            nc.scalar.activation(std_sb, ph3_s, AF.Identity, bias=bcol(27, rows=NA))
            nc.sync.dma_start(out=std_d[:], in_=std_sb)

    _split_excess_waits(nc)
    return nc


def prep_weights(inp):
    """Host-side weight preprocessing -> dict of replicated arrays."""
    f = lambda a: np.ascontiguousarray(a, dtype=np.float32)
    e1_w = np.asarray(inp["e1_w"], np.float32)
    n1_w = np.asarray(inp["n1_w"], np.float32)
    ln_g = np.asarray(inp["ln_g"], np.float32)
    ln_b = np.asarray(inp["ln_b"], np.float32)
    n2_b = np.asarray(inp["n2_b"], np.float32)
    mu1_w = np.asarray(inp["mu1_w"], np.float32)
    s1_w = np.asarray(inp["s1_w"], np.float32)

    d = {}
    d["e1AT"] = f(e1_w[:, :D].T)
    d["e1CT"] = f(e1_w[:, D:].T)
    edge_np = np.float32
    if EDGE_DT == "bf16":
        import ml_dtypes
        edge_np = ml_dtypes.bfloat16
    d["e2T"] = np.ascontiguousarray(
        np.asarray(inp["e2_w"], np.float32).T.reshape(FT, P, H), dtype=edge_np)
    d["e3T"] = np.ascontiguousarray(
        np.asarray(inp["e3_w"], np.float32).T.reshape(FT, P, H), dtype=edge_np)
    d["n1aT"] = f((n1_w[:, D : D + H] * ln_g[None, :]).T.reshape(FT, P, H))
    d["n1oT"] = f(n1_w[:, :D].T)
    d["n1sT"] = f(n1_w[:, D + H :].T)
    d["n2T"] = f(np.asarray(inp["n2_w"], np.float32).T.reshape(FT, P, HH))
    d["layerT"] = f(np.asarray(inp["layer_w"], np.float32).T)

    mu1 = mu1_w.copy()
    mu1[:, H : H + HH] *= 1.0 / K
    d["mu1T"] = f(mu1.T.reshape(2 * FT, P, 256))
    s1 = s1_w.copy()
    s1[:, H : H + HH] *= 1.0 / K
    d["s1T"] = f(s1.T.reshape(2 * FT, P, 256))
    d["mu2T"] = f(np.asarray(inp["mu2_w"], np.float32).T.reshape(2, P, 128))
    d["s2T"] = f(np.asarray(inp["s2_w"], np.float32).T.reshape(2, P, 128))
    d["mu3T"] = f(np.asarray(inp["mu3_w"], np.float32).T)
    d["s3T"] = f(np.asarray(inp["s3_w"], np.float32).T)

    n1_b_eff = np.asarray(inp["n1_b"], np.float32) + n1_w[:, D : D + H] @ ln_b
    mu1_b_eff = (np.asarray(inp["mu1_b"], np.float32)
                 + (mu1_w[:, H : H + HH] + mu1_w[:, H + HH :]) @ n2_b)
    s1_b_eff = (np.asarray(inp["s1_b"], np.float32)
                + (s1_w[:, H : H + HH] + s1_w[:, H + HH :]) @ n2_b)

    bp = np.zeros((P, 32), np.float32)
    bp[:, 0:4] = np.asarray(inp["e1_b"], np.float32).reshape(FT, P).T
    bp[:, 4:8] = np.asarray(inp["e2_b"], np.float32).reshape(FT, P).T
    bp[:, 8:12] = np.asarray(inp["e3_b"], np.float32).reshape(FT, P).T
    bp[:, 12:16] = n1_b_eff.reshape(FT, P).T
    bp[:, 16:20] = np.asarray(inp["layer_b"], np.float32).reshape(FT, P).T
    bp[:, 20:22] = mu1_b_eff.reshape(2, P).T
    bp[:, 22] = np.asarray(inp["mu2_b"], np.float32)
    bp[0:NA, 23] = np.asarray(inp["mu3_b"], np.float32)
    bp[:, 24:26] = s1_b_eff.reshape(2, P).T
    bp[:, 26] = np.asarray(inp["s2_b"], np.float32)
    bp[0:NA, 27] = np.asarray(inp["s3_b"], np.float32)
    d["bias_pack"] = bp
    return d


def make_in_maps(inputs):
    w = prep_weights(inputs)
    obs = np.ascontiguousarray(np.asarray(inputs["obs"], np.float32))
    state = np.asarray(inputs["state"], np.float32)
    in_maps = []
    for c in range(NCORES):
        m = dict(w)
        m["obs"] = np.ascontiguousarray(obs[c * BSH : (c + 1) * BSH])
        m["stateT"] = np.ascontiguousarray(state[c * BSH : (c + 1) * BSH].T)
        in_maps.append(m)
    return in_maps


_NC_CACHE = {}


def get_nc():
    key = (EDGE_DT, SMALL_DT)
    if key not in _NC_CACHE:
        _NC_CACHE[key] = build_bass()
    return _NC_CACHE[key]


def run(in_maps, trace=False, **kw):
    nc = get_nc()
    return run_bass_kernel_spmd(nc, in_maps, core_ids=list(range(NCORES)),
                                trace=trace, **kw)


def gather(res_list):
    mu = np.concatenate([r["mu"].T for r in res_list], axis=0)
    pre = np.concatenate([r["std"].T for r in res_list], axis=0).astype(np.float64)
    std = np.clip(np.log1p(np.exp(pre)) + 0.001, 0.1, 2.0)
    return mu.astype(np.float32), std.astype(np.float32)


def kernel(**inputs):
    res = run(make_in_maps(inputs))
    return gather(res.results)


# revision 7
# speedup vs baseline: 1.0160x; 1.0160x over previous
"""Trainium2 Bass kernel for nn_ActorNetwork (GNN message passing actor).

Self-contained: hardcodes shapes B=32, K=64, D=4, DS=4, H=512, HH=256, NA=2.
Data-parallel over batch across 8 NeuronCores (4 samples/core), all params
replicated. Returns (mu, std) like the reference.

Host-side weight prep folds: LayerNorm affine into n1, the /counts
normalizations into the LN eps and the head-1 weights, n2 bias into the
head-1 biases. The edge-MLP first layer is split into U = A@o_i + b and
V = C@o_j so the [K,K,2D] edge input tensor is never materialized.
"""
import numpy as np

import concourse.bass as bass
import concourse.mybir as mybir
from concourse.bass_utils import run_bass_kernel_spmd
from concourse.tile import TileContext

# ---- problem constants ----
B, K, D, DS, H, HH, NA = 32, 64, 4, 4, 512, 256, 2
NCORES = 8
BSH = B // NCORES            # samples per core = 4
P = 128
FT = H // P                  # 4 feature tiles of hidden dim
TOK = BSH * K                # 256 node tokens per core
IBLK = 8                     # i-rows per edge chunk (8*64 = 512 tokens)
NCH = K // IBLK              # 8 chunks per sample
EPS_S = (K * K) * 1e-5       # LN eps scaled for un-normalized agg sums

F32 = mybir.dt.float32
F32R = mybir.dt.float32r
BF16 = mybir.dt.bfloat16
AF = mybir.ActivationFunctionType
ALU = mybir.AluOpType

# dtype knobs: 'f32' | 'f32r' | 'bf16' for the big edge matmuls,
# 'f32' | 'f32r' for the small matmuls elsewhere.
import os as _os

EDGE_DT = _os.environ.get("KERNEL_EDGE_DT", "f32")
SMALL_DT = _os.environ.get("KERNEL_SMALL_DT", "f32")


def _mm(ap, kind):
    if kind == "f32r" and ap.dtype == F32:
        return ap.bitcast(F32R)
    return ap


def _split_excess_waits(nc, max_waits=1):
    """walrus in this container rejects >~2 sem waits on one instruction
    (hits the Tile tail drain). Split excess waits onto same-engine NoOps."""
    for f in nc.m.functions:
        for bb in f.blocks:
            insts = list(bb.instructions)
            new_list = []
            changed = False
            for inst in insts:
                si = inst.sync_info
                if si is not None and si.on_wait and len(si.on_wait) > max_waits:
                    waits = list(si.on_wait)
                    extra, keep = waits[:-max_waits], waits[-max_waits:]
                    for k0 in range(0, len(extra), max_waits):
                        chunk = extra[k0 : k0 + max_waits]
                        nop = mybir.InstNoOp(
                            name=f"{inst.name}-wsplit-{k0}",
                            engine=inst.engine,
                            ins=[],
                            outs=[],
                            sync_info=mybir.SyncInfo(on_wait=chunk, on_update=[]),
                        )
                        new_list.append(nop)
                        changed = True
                    si.on_wait = keep
                new_list.append(inst)
            if changed:
                bb.instructions = new_list


def build_bass():
    edge_store = BF16 if EDGE_DT == "bf16" else F32
    nc = bass.Bass("TRN2", debug=False, num_devices=NCORES)

    def dp(nm, sh, dt=F32):
        return nc.declare_dram_parameter(nm, sh, dt, isOutput=False)

    obs_d = dp("obs", [BSH, D, K])
    st_d = dp("stateT", [DS, BSH])
    e1AT_d = dp("e1AT", [D, H])
    e1CT_d = dp("e1CT", [D, H])
    e2T_d = dp("e2T", [FT, P, H], edge_store)
    e3T_d = dp("e3T", [FT, P, H], edge_store)
    n1aT_d = dp("n1aT", [FT, P, H])
    n1oT_d = dp("n1oT", [D, H])
    n1sT_d = dp("n1sT", [DS, H])
    n2T_d = dp("n2T", [FT, P, HH])
    layerT_d = dp("layerT", [DS, H])
    mu1T_d = dp("mu1T", [2 * FT, P, 256])
    s1T_d = dp("s1T", [2 * FT, P, 256])
    mu2T_d = dp("mu2T", [2, P, 128])
    s2T_d = dp("s2T", [2, P, 128])
    mu3T_d = dp("mu3T", [P, NA])
    s3T_d = dp("s3T", [P, NA])
    bias_d = dp("bias_pack", [P, 32])
    mu_d = nc.declare_dram_parameter("mu", [NA, BSH], F32, isOutput=True)
    std_d = nc.declare_dram_parameter("std", [NA, BSH], F32, isOutput=True)

    with TileContext(nc) as tc:
        with (
            tc.tile_pool(name="w", bufs=1) as wp,
            tc.tile_pool(name="act", bufs=1) as pa,
            tc.tile_pool(name="chunk", bufs=2) as cp,
            tc.tile_pool(name="ps", bufs=8, space="PSUM") as pp,
        ):
            # ---------------- weight loads ----------------
            def wload(nm, dram, idx=None, dt=F32):
                src = dram[:] if idx is None else dram[idx]
                t = wp.tile(list(src.shape), dt, name=nm, tag=nm)
                nc.sync.dma_start(out=t, in_=src)
                return t

            e2w = [wload(f"e2w{k}", e2T_d, k, edge_store) for k in range(FT)]
            e3w = [wload(f"e3w{k}", e3T_d, k, edge_store) for k in range(FT)]
            e1Aw = wload("e1Aw", e1AT_d)
            e1Cw = wload("e1Cw", e1CT_d)
            n1aw = [wload(f"n1aw{k}", n1aT_d, k) for k in range(FT)]
            n1ow = wload("n1ow", n1oT_d)
            n1sw = wload("n1sw", n1sT_d)
            n2w = [wload(f"n2w{k}", n2T_d, k) for k in range(FT)]
            layerw = wload("layerw", layerT_d)
            mu1w = [wload(f"mu1w{k}", mu1T_d, k) for k in range(2 * FT)]
            s1w = [wload(f"s1w{k}", s1T_d, k) for k in range(2 * FT)]
            mu2w = [wload(f"mu2w{k}", mu2T_d, k) for k in range(2)]
            s2w = [wload(f"s2w{k}", s2T_d, k) for k in range(2)]
            mu3w = wload("mu3w", mu3T_d)
            s3w = wload("s3w", s3T_d)
            bias_t = wload("bias_t", bias_d)

            def bcol(i, rows=P):
                return bias_t[0:rows, i : i + 1]

            # ---------------- activations in ----------------
            o_all = pa.tile([D, TOK], F32, name="o_all", tag="o_all")
            nc.sync.dma_start(
                out=o_all[:].rearrange("d (b k) -> d b k", b=BSH),
                in_=obs_d[:].rearrange("b d k -> d b k"),
            )
            st_t = pa.tile([DS, BSH], F32, name="st_t", tag="st_t")
            nc.sync.dma_start(out=st_t, in_=st_d[:])
            state_bc = pa.tile([DS, TOK], F32, name="state_bc", tag="state_bc")
            nc.vector.tensor_copy(
                state_bc[:].rearrange("s (b k) -> s b k", b=BSH),
                st_t[:, :, None].broadcast_to([DS, BSH, K]),
            )

            # ---------------- U/V (edge layer 1, split) ----------------
            U_all, V_all, agg = [], [], []
            for m in range(FT):
                msl = slice(m * P, (m + 1) * P)
                pu = pp.tile([P, TOK], F32, name=f"pu{m}", tag="ps")
                nc.tensor.matmul(
                    pu, _mm(e1Aw[:, msl], SMALL_DT), _mm(o_all, SMALL_DT),
                    start=True, stop=True,
                )
                Um = pa.tile([P, TOK], F32, name=f"U{m}", tag=f"U{m}")
                nc.scalar.activation(Um, pu, AF.Identity, bias=bcol(0 + m))
                U_all.append(Um)

                pv = pp.tile([P, TOK], F32, name=f"pv{m}", tag="ps")
                nc.tensor.matmul(
                    pv, _mm(e1Cw[:, msl], SMALL_DT), _mm(o_all, SMALL_DT),
                    start=True, stop=True,
                )
                Vm = pa.tile([P, TOK], F32, name=f"V{m}", tag=f"V{m}")
                nc.vector.tensor_copy(Vm, pv)
                V_all.append(Vm)

                am = pa.tile([P, TOK], F32, name=f"agg{m}", tag=f"agg{m}")
                agg.append(am)

            # ---------------- edge MLP over K x K pairs ----------------
            for b in range(BSH):
                for ib in range(NCH):
                    i0 = b * K + ib * IBLK
                    h1 = []
                    for m in range(FT):
                        h1m = cp.tile([P, IBLK * K], edge_store,
                                      name=f"h1_{m}", tag=f"h1_{m}")
                        Ubc = U_all[m][:, i0 : i0 + IBLK][:, :, None].broadcast_to(
                            [P, IBLK, K])
                        Vbc = V_all[m][:, b * K : (b + 1) * K][:, None, :].broadcast_to(
                            [P, IBLK, K])
                        nc.vector.tensor_add(
                            h1m[:].rearrange("p (i j) -> p i j", i=IBLK), Ubc, Vbc)
                        nc.gpsimd.tensor_relu(h1m, h1m)
                        h1.append(h1m)
                    h2 = []
                    for m in range(FT):
                        msl = slice(m * P, (m + 1) * P)
                        ps2 = pp.tile([P, IBLK * K], F32, name=f"ps2_{m}", tag="ps")
                        for k2 in range(FT):
                            nc.tensor.matmul(
                                ps2, _mm(e2w[k2][:, msl], EDGE_DT),
                                _mm(h1[k2], EDGE_DT),
                                start=(k2 == 0), stop=(k2 == FT - 1),
                            )
                        h2m = cp.tile([P, IBLK * K], edge_store,
                                      name=f"h2_{m}", tag=f"h2_{m}")
                        nc.scalar.activation(h2m, ps2, AF.Relu, bias=bcol(4 + m))
                        h2.append(h2m)
                    for m in range(FT):
                        msl = slice(m * P, (m + 1) * P)
                        ps3 = pp.tile([P, IBLK * K], F32, name=f"ps3_{m}", tag="ps")
                        for k2 in range(FT):
                            nc.tensor.matmul(
                                ps3, _mm(e3w[k2][:, msl], EDGE_DT),
                                _mm(h2[k2], EDGE_DT),
                                start=(k2 == 0), stop=(k2 == FT - 1),
                            )
                        h3m = cp.tile([P, IBLK * K], edge_store,
                                      name=f"h3_{m}", tag=f"h3_{m}")
                        nc.scalar.activation(h3m, ps3, AF.Relu, bias=bcol(8 + m))
                        nc.vector.reduce_sum(
                            out=agg[m][:, i0 : i0 + IBLK],
                            in_=h3m[:].rearrange("p (i j) -> p i j", i=IBLK),
                            axis=mybir.AxisListType.X,
                        )

            # ---------------- LayerNorm over H (token stats via matmul) ----
            ones_col = pa.tile([P, 1], F32, name="ones_col", tag="ones_col")
            nc.vector.memset(ones_col, 1.0)
            ones_row = pa.tile([1, P], F32, name="ones_row", tag="ones_row")
            nc.vector.memset(ones_row, 1.0)

            sq = []
            for m in range(FT):
                sqm = pa.tile([P, TOK], F32, name=f"sq{m}", tag=f"sq{m}")
                nc.vector.tensor_mul(sqm, agg[m], agg[m])
                sq.append(sqm)
            ps_sum = pp.tile([1, TOK], F32, name="ps_sum", tag="ps")
            ps_ssq = pp.tile([1, TOK], F32, name="ps_ssq", tag="ps")
            for m in range(FT):
                nc.tensor.matmul(ps_sum, _mm(ones_col, SMALL_DT),
                                 _mm(agg[m], SMALL_DT),
                                 start=(m == 0), stop=(m == FT - 1))
            for m in range(FT):
                nc.tensor.matmul(ps_ssq, _mm(ones_col, SMALL_DT),
                                 _mm(sq[m], SMALL_DT),
                                 start=(m == 0), stop=(m == FT - 1))
            mean_r = pa.tile([1, TOK], F32, name="mean_r", tag="mean_r")
            nc.vector.tensor_scalar_mul(mean_r, ps_sum, 1.0 / H)
            msq_r = pa.tile([1, TOK], F32, name="msq_r", tag="msq_r")
            nc.vector.tensor_mul(msq_r, mean_r, mean_r)
            var_r = pa.tile([1, TOK], F32, name="var_r", tag="var_r")
            nc.vector.scalar_tensor_tensor(
                var_r, ps_ssq, 1.0 / H, msq_r, op0=ALU.mult, op1=ALU.subtract)
            eps_t = pa.tile([1, 1], F32, name="eps_t", tag="eps_t")
            nc.vector.memset(eps_t, EPS_S)
            sd_r = pa.tile([1, TOK], F32, name="sd_r", tag="sd_r")
            nc.scalar.activation(sd_r, var_r, AF.Sqrt, bias=eps_t)
            rstd_r = pa.tile([1, TOK], F32, name="rstd_r", tag="rstd_r")
            nc.vector.reciprocal(rstd_r, sd_r)

            ps_mb = pp.tile([P, TOK], F32, name="ps_mb", tag="ps")
            nc.tensor.matmul(ps_mb, _mm(ones_row, SMALL_DT),
                             _mm(mean_r, SMALL_DT), start=True, stop=True)
            mean_bc = pa.tile([P, TOK], F32, name="mean_bc", tag="mean_bc")
            nc.scalar.copy(mean_bc, ps_mb)
            ps_rb = pp.tile([P, TOK], F32, name="ps_rb", tag="ps")
            nc.tensor.matmul(ps_rb, _mm(ones_row, SMALL_DT),
                             _mm(rstd_r, SMALL_DT), start=True, stop=True)
            rstd_bc = pa.tile([P, TOK], F32, name="rstd_bc", tag="rstd_bc")
            nc.scalar.copy(rstd_bc, ps_rb)

            aggn = []
            for m in range(FT):
                anm = pa.tile([P, TOK], F32, name=f"aggn{m}", tag=f"aggn{m}")
                nc.vector.tensor_sub(anm, agg[m], mean_bc)
                nc.vector.tensor_mul(anm, anm, rstd_bc)
                aggn.append(anm)

            # ---------------- node MLP ----------------
            hn1 = []
            for m in range(FT):
                msl = slice(m * P, (m + 1) * P)
                psn = pp.tile([P, TOK], F32, name=f"psn1_{m}", tag="ps")
                for k2 in range(FT):
                    nc.tensor.matmul(psn, _mm(n1aw[k2][:, msl], SMALL_DT),
                                     _mm(aggn[k2], SMALL_DT),
                                     start=(k2 == 0), stop=False)
                nc.tensor.matmul(psn, _mm(n1ow[:, msl], SMALL_DT),
                                 _mm(o_all, SMALL_DT), start=False, stop=False)
                nc.tensor.matmul(psn, _mm(n1sw[:, msl], SMALL_DT),
                                 _mm(state_bc, SMALL_DT), start=False, stop=True)
                hm = pa.tile([P, TOK], F32, name=f"hn1_{m}", tag=f"hn1_{m}")
                nc.scalar.activation(hm, psn, AF.Relu, bias=bcol(12 + m))
                hn1.append(hm)

            pool_sum, pool_max = [], []
            for m2 in range(HH // P):
                msl = slice(m2 * P, (m2 + 1) * P)
                psn2 = pp.tile([P, TOK], F32, name=f"psn2_{m2}", tag="ps")
                for k2 in range(FT):
                    nc.tensor.matmul(psn2, _mm(n2w[k2][:, msl], SMALL_DT),
                                     _mm(hn1[k2], SMALL_DT),
                                     start=(k2 == 0), stop=(k2 == FT - 1))
                sm = pa.tile([P, BSH], F32, name=f"pool_s{m2}", tag=f"pool_s{m2}")
                nc.vector.reduce_sum(
                    out=sm, in_=psn2[:].rearrange("p (b k) -> p b k", b=BSH),
                    axis=mybir.AxisListType.X)
                mm_ = pa.tile([P, BSH], F32, name=f"pool_m{m2}", tag=f"pool_m{m2}")
                nc.vector.reduce_max(
                    out=mm_, in_=psn2[:].rearrange("p (b k) -> p b k", b=BSH),
                    axis=mybir.AxisListType.X)
                pool_sum.append(sm)
                pool_max.append(mm_)

            # ---------------- heads ----------------
            xs = []
            for m in range(FT):
                msl = slice(m * P, (m + 1) * P)
                pst = pp.tile([P, BSH], F32, name=f"pst{m}", tag="ps")
                nc.tensor.matmul(pst, _mm(layerw[:, msl], SMALL_DT),
                                 _mm(st_t, SMALL_DT), start=True, stop=True)
                xm = pa.tile([P, BSH], F32, name=f"xst{m}", tag=f"xst{m}")
                nc.scalar.activation(xm, pst, AF.Relu, bias=bcol(16 + m))
                xs.append(xm)
            xs += pool_sum + pool_max  # x = [st_feat, mean_pool, max_pool]

            def head(w1, w2, w3, bc1, bc2, tag):
                hl1 = []
                for m in range(2):
                    msl = slice(m * P, (m + 1) * P)
                    ph = pp.tile([P, BSH], F32, name=f"p{tag}1_{m}", tag="ps")
                    for k2 in range(2 * FT):
                        nc.tensor.matmul(ph, _mm(w1[k2][:, msl], SMALL_DT),
                                         _mm(xs[k2], SMALL_DT),
                                         start=(k2 == 0), stop=(k2 == 2 * FT - 1))
                    hm = pa.tile([P, BSH], F32, name=f"h{tag}1_{m}",
                                 tag=f"h{tag}1_{m}")
                    nc.scalar.activation(hm, ph, AF.Relu, bias=bcol(bc1 + m))
                    hl1.append(hm)
                ph2 = pp.tile([P, BSH], F32, name=f"p{tag}2", tag="ps")
                for k2 in range(2):
                    nc.tensor.matmul(ph2, _mm(w2[k2], SMALL_DT),
                                     _mm(hl1[k2], SMALL_DT),
                                     start=(k2 == 0), stop=(k2 == 1))
                hm2 = pa.tile([P, BSH], F32, name=f"h{tag}2", tag=f"h{tag}2")
                nc.scalar.activation(hm2, ph2, AF.Relu, bias=bcol(bc2))
                ph3 = pp.tile([NA, BSH], F32, name=f"p{tag}3", tag="ps")
                nc.tensor.matmul(ph3, _mm(w3, SMALL_DT), _mm(hm2, SMALL_DT),
                                 start=True, stop=True)
                return ph3

            ph3_mu = head(mu1w, mu2w, mu3w, 20, 22, "mu")
            mu_sb = pa.tile([NA, BSH], F32, name="mu_sb", tag="mu_sb")
            nc.scalar.activation(mu_sb, ph3_mu, AF.Identity, bias=bcol(23, rows=NA))
            nc.sync.dma_start(out=mu_d[:], in_=mu_sb)

            # softplus isn't co-loadable with sqrt in the ACT func sets;
            # emit the pre-softplus logits, host applies softplus+clip.
            ph3_s = head(s1w, s2w, s3w, 24, 26, "s")
            std_sb = pa.tile([NA, BSH], F32, name="std_sb", tag="std_sb")
            nc.scalar.activation(std_sb, ph3_s, AF.Identity, bias=bcol(27, rows=NA))
            nc.sync.dma_start(out=std_d[:], in_=std_sb)

    _split_excess_waits(nc)
    return nc


def prep_weights(inp):
    """Host-side weight preprocessing -> dict of replicated arrays."""
    f = lambda a: np.ascontiguousarray(a, dtype=np.float32)
    e1_w = np.asarray(inp["e1_w"], np.float32)
    n1_w = np.asarray(inp["n1_w"], np.float32)
    ln_g = np.asarray(inp["ln_g"], np.float32)
    ln_b = np.asarray(inp["ln_b"], np.float32)
    n2_b = np.asarray(inp["n2_b"], np.float32)
    mu1_w = np.asarray(inp["mu1_w"], np.float32)
    s1_w = np.asarray(inp["s1_w"], np.float32)

    d = {}
    d["e1AT"] = f(e1_w[:, :D].T)
    d["e1CT"] = f(e1_w[:, D:].T)
    edge_np = np.float32
    if EDGE_DT == "bf16":
        import ml_dtypes
        edge_np = ml_dtypes.bfloat16
    d["e2T"] = np.ascontiguousarray(
        np.asarray(inp["e2_w"], np.float32).T.reshape(FT, P, H), dtype=edge_np)
    d["e3T"] = np.ascontiguousarray(
        np.asarray(inp["e3_w"], np.float32).T.reshape(FT, P, H), dtype=edge_np)
    d["n1aT"] = f((n1_w[:, D : D + H] * ln_g[None, :]).T.reshape(FT, P, H))
    d["n1oT"] = f(n1_w[:, :D].T)
    d["n1sT"] = f(n1_w[:, D + H :].T)
    d["n2T"] = f(np.asarray(inp["n2_w"], np.float32).T.reshape(FT, P, HH))
    d["layerT"] = f(np.asarray(inp["layer_w"], np.float32).T)

    mu1 = mu1_w.copy()
    mu1[:, H : H + HH] *= 1.0 / K
    d["mu1T"] = f(mu1.T.reshape(2 * FT, P, 256))
    s1 = s1_w.copy()
    s1[:, H : H + HH] *= 1.0 / K
    d["s1T"] = f(s1.T.reshape(2 * FT, P, 256))
    d["mu2T"] = f(np.asarray(inp["mu2_w"], np.float32).T.reshape(2, P, 128))
    d["s2T"] = f(np.asarray(inp["s2_w"], np.float32).T.reshape(2, P, 128))
    d["mu3T"] = f(np.asarray(inp["mu3_w"], np.float32).T)
    d["s3T"] = f(np.asarray(inp["s3_w"], np.float32).T)

    n1_b_eff = np.asarray(inp["n1_b"], np.float32) + n1_w[:, D : D + H] @ ln_b
    mu1_b_eff = (np.asarray(inp["mu1_b"], np.float32)
                 + (mu1_w[:, H : H + HH] + mu1_w[:, H + HH :]) @ n2_b)
    s1_b_eff = (np.asarray(inp["s1_b"], np.float32)
                + (s1_w[:, H : H + HH] + s1_w[:, H + HH :]) @ n2_b)

    bp = np.zeros((P, 32), np.float32)
    bp[:, 0:4] = np.asarray(inp["e1_b"], np.float32).reshape(FT, P).T
    bp[:, 4:8] = np.asarray(inp["e2_b"], np.float32).reshape(FT, P).T
    bp[:, 8:12] = np.asarray(inp["e3_b"], np.float32).reshape(FT, P).T
    bp[:, 12:16] = n1_b_eff.reshape(FT, P).T
    bp[:, 16:20] = np.asarray(inp["layer_b"], np.float32).reshape(FT, P).T
    bp[:, 20:22] = mu1_b_eff.reshape(2, P).T
    bp[:, 22] = np.asarray(inp["mu2_b"], np.float32)
    bp[0:NA, 23] = np.asarray(inp["mu3_b"], np.float32)
    bp[:, 24:26] = s1_b_eff.reshape(2, P).T
    bp[:, 26] = np.asarray(inp["s2_b"], np.float32)
    bp[0:NA, 27] = np.asarray(inp["s3_b"], np.float32)
    d["bias_pack"] = bp
    return d


def make_in_maps(inputs):
    w = prep_weights(inputs)
    obs = np.ascontiguousarray(np.asarray(inputs["obs"], np.float32))
    state = np.asarray(inputs["state"], np.float32)
    in_maps = []
    for c in range(NCORES):
        m = dict(w)
        m["obs"] = np.ascontiguousarray(obs[c * BSH : (c + 1) * BSH])
        m["stateT"] = np.ascontiguousarray(state[c * BSH : (c + 1) * BSH].T)
        in_maps.append(m)
    return in_maps


_NC_CACHE = {}


def get_nc():
    key = (EDGE_DT, SMALL_DT)
    if key not in _NC_CACHE:
        _NC_CACHE[key] = build_bass()
    return _NC_CACHE[key]


def run(in_maps, trace=False, **kw):
    nc = get_nc()
    return run_bass_kernel_spmd(nc, in_maps, core_ids=list(range(NCORES)),
                                trace=trace, **kw)


def gather(res_list):
    mu = np.concatenate([r["mu"].T for r in res_list], axis=0)
    pre = np.concatenate([r["std"].T for r in res_list], axis=0).astype(np.float64)
    std = np.clip(np.log1p(np.exp(pre)) + 0.001, 0.1, 2.0)
    return mu.astype(np.float32), std.astype(np.float32)


def kernel(**inputs):
    res = run(make_in_maps(inputs))
    return gather(res.results)


# revision 11
# speedup vs baseline: 3.3031x; 3.2513x over previous
"""Trainium2 Bass kernel for nn_ActorNetwork (GNN message passing actor).

Self-contained: hardcodes shapes B=32, K=64, D=4, DS=4, H=512, HH=256, NA=2.
Data-parallel over batch across 8 NeuronCores (4 samples/core), all params
replicated. Returns (mu, std) like the reference.

Host-side weight prep folds: LayerNorm affine into n1, the /counts
normalizations into the LN eps and the head-1 weights, n2 bias into the
head-1 biases. The edge-MLP first layer is split into U = A@o_i + b and
V = C@o_j so the [K,K,2D] edge input tensor is never materialized.
"""
import numpy as np

import concourse.bass as bass
import concourse.mybir as mybir
from concourse.bass_utils import run_bass_kernel_spmd
from concourse.tile import TileContext

# ---- problem constants ----
B, K, D, DS, H, HH, NA = 32, 64, 4, 4, 512, 256, 2
NCORES = 8
BSH = B // NCORES            # samples per core = 4
P = 128
FT = H // P                  # 4 feature tiles of hidden dim
TOK = BSH * K                # 256 node tokens per core
IBLK = 8                     # i-rows per edge chunk (8*64 = 512 tokens)
NCH = K // IBLK              # 8 chunks per sample
EPS_S = (K * K) * 1e-5       # LN eps scaled for un-normalized agg sums

F32 = mybir.dt.float32
F32R = mybir.dt.float32r
BF16 = mybir.dt.bfloat16
AF = mybir.ActivationFunctionType
ALU = mybir.AluOpType

# dtype knobs: 'f32' | 'f32r' | 'bf16' for the big edge matmuls,
# 'f32' | 'f32r' for the small matmuls elsewhere.
import os as _os

EDGE_DT = _os.environ.get("KERNEL_EDGE_DT", "f32")
SMALL_DT = _os.environ.get("KERNEL_SMALL_DT", "f32")


def _mm(ap, kind):
    if kind == "f32r" and ap.dtype == F32:
        return ap.bitcast(F32R)
    return ap


def _split_excess_waits(nc, max_waits=1):
    """walrus in this container rejects >~2 sem waits on one instruction
    (hits the Tile tail drain). Split excess waits onto same-engine NoOps."""
    for f in nc.m.functions:
        for bb in f.blocks:
            insts = list(bb.instructions)
            new_list = []
            changed = False
            for inst in insts:
                si = inst.sync_info
                if si is not None and si.on_wait and len(si.on_wait) > max_waits:
                    waits = list(si.on_wait)
                    extra, keep = waits[:-max_waits], waits[-max_waits:]
                    for k0 in range(0, len(extra), max_waits):
                        chunk = extra[k0 : k0 + max_waits]
                        nop = mybir.InstNoOp(
                            name=f"{inst.name}-wsplit-{k0}",
                            engine=inst.engine,
                            ins=[],
                            outs=[],
                            sync_info=mybir.SyncInfo(on_wait=chunk, on_update=[]),
                        )
                        new_list.append(nop)
                        changed = True
                    si.on_wait = keep
                new_list.append(inst)
            if changed:
                bb.instructions = new_list


def build_bass():
    edge_store = BF16 if EDGE_DT == "bf16" else F32
    nc = bass.Bass("TRN2", debug=False, num_devices=NCORES)

    def dp(nm, sh, dt=F32):
        return nc.declare_dram_parameter(nm, sh, dt, isOutput=False)

    obs_d = dp("obs", [BSH, D, K])
    st_d = dp("stateT", [DS, BSH])
    e1AT_d = dp("e1AT", [D, H])
    e1CT_d = dp("e1CT", [D, H])
    e2T_d = dp("e2T", [FT, P, H], edge_store)
    e3T_d = dp("e3T", [FT, P, H], edge_store)
    n1aT_d = dp("n1aT", [FT, P, H])
    n1oT_d = dp("n1oT", [D, H])
    n1sT_d = dp("n1sT", [DS, H])
    n2T_d = dp("n2T", [FT, P, HH])
    layerT_d = dp("layerT", [DS, H])
    mu1T_d = dp("mu1T", [2 * FT, P, 256])
    s1T_d = dp("s1T", [2 * FT, P, 256])
    mu2T_d = dp("mu2T", [2, P, 128])
    s2T_d = dp("s2T", [2, P, 128])
    mu3T_d = dp("mu3T", [P, NA])
    s3T_d = dp("s3T", [P, NA])
    bias_d = dp("bias_pack", [P, 32])
    mu_d = nc.declare_dram_parameter("mu", [NA, BSH], F32, isOutput=True)
    std_d = nc.declare_dram_parameter("std", [NA, BSH], F32, isOutput=True)

    with TileContext(nc) as tc:
        with (
            tc.tile_pool(name="w", bufs=1) as wp,
            tc.tile_pool(name="act", bufs=1) as pa,
            tc.tile_pool(name="chunk", bufs=2) as cp,
            tc.tile_pool(name="ps", bufs=8, space="PSUM") as pp,
        ):
            # ---------------- weight loads ----------------
            def wload(nm, dram, idx=None, dt=F32):
                src = dram[:] if idx is None else dram[idx]
                t = wp.tile(list(src.shape), dt, name=nm, tag=nm)
                nc.sync.dma_start(out=t, in_=src)
                return t

            e2w = [wload(f"e2w{k}", e2T_d, k, edge_store) for k in range(FT)]
            e3w = [wload(f"e3w{k}", e3T_d, k, edge_store) for k in range(FT)]
            e1Aw = wload("e1Aw", e1AT_d)
            e1Cw = wload("e1Cw", e1CT_d)
            n1aw = [wload(f"n1aw{k}", n1aT_d, k) for k in range(FT)]
            n1ow = wload("n1ow", n1oT_d)
            n1sw = wload("n1sw", n1sT_d)
            n2w = [wload(f"n2w{k}", n2T_d, k) for k in range(FT)]
            layerw = wload("layerw", layerT_d)
            mu1w = [wload(f"mu1w{k}", mu1T_d, k) for k in range(2 * FT)]
            s1w = [wload(f"s1w{k}", s1T_d, k) for k in range(2 * FT)]
            mu2w = [wload(f"mu2w{k}", mu2T_d, k) for k in range(2)]
            s2w = [wload(f"s2w{k}", s2T_d, k) for k in range(2)]
            mu3w = wload("mu3w", mu3T_d)
            s3w = wload("s3w", s3T_d)
            bias_t = wload("bias_t", bias_d)

            def bcol(i, rows=P):
                return bias_t[0:rows, i : i + 1]

            # ---------------- activations in ----------------
            o_all = pa.tile([D, TOK], F32, name="o_all", tag="o_all")
            nc.sync.dma_start(
                out=o_all[:].rearrange("d (b k) -> d b k", b=BSH),
                in_=obs_d[:].rearrange("b d k -> d b k"),
            )
            st_t = pa.tile([DS, BSH], F32, name="st_t", tag="st_t")
            nc.sync.dma_start(out=st_t, in_=st_d[:])
            state_bc = pa.tile([DS, TOK], F32, name="state_bc", tag="state_bc")
            nc.vector.tensor_copy(
                state_bc[:].rearrange("s (b k) -> s b k", b=BSH),
                st_t[:, :, None].broadcast_to([DS, BSH, K]),
            )

            # ---------------- U/V (edge layer 1, split) ----------------
            U_all, V_all, agg = [], [], []
            for m in range(FT):
                msl = slice(m * P, (m + 1) * P)
                pu = pp.tile([P, TOK], F32, name=f"pu{m}", tag="ps")
                nc.tensor.matmul(
                    pu, _mm(e1Aw[:, msl], SMALL_DT), _mm(o_all, SMALL_DT),
                    start=True, stop=True,
                )
                # U stays f32: tensor_scalar scalar operands must be float32
                Um = pa.tile([P, TOK], F32, name=f"U{m}", tag=f"U{m}")
                nc.scalar.activation(Um, pu, AF.Identity, bias=bcol(0 + m))
                U_all.append(Um)

                pv = pp.tile([P, TOK], F32, name=f"pv{m}", tag="ps")
                nc.tensor.matmul(
                    pv, _mm(e1Cw[:, msl], SMALL_DT), _mm(o_all, SMALL_DT),
                    start=True, stop=True,
                )
                Vm = pa.tile([P, TOK], edge_store, name=f"V{m}", tag=f"V{m}")
                nc.vector.tensor_copy(Vm, pv)
                V_all.append(Vm)

                am = pa.tile([P, TOK], F32, name=f"agg{m}", tag=f"agg{m}")
                agg.append(am)

            # ---------------- edge MLP over K x K pairs ----------------
            for b in range(BSH):
                for ib in range(NCH):
                    i0 = b * K + ib * IBLK
                    h1 = []
                    for m in range(FT):
                        h1m = cp.tile([P, IBLK * K], edge_store,
                                      name=f"h1_{m}", tag=f"h1_{m}")
                        Vsl = V_all[m][:, b * K : (b + 1) * K]
                        for i in range(IBLK):
                            # h1[:, i*K:(i+1)*K] = relu(V_j + U_i)
                            nc.vector.tensor_scalar(
                                h1m[:, i * K : (i + 1) * K], Vsl,
                                U_all[m][:, i0 + i : i0 + i + 1], 0.0,
                                op0=ALU.add, op1=ALU.max)
                        h1.append(h1m)
                    h2 = []
                    for m in range(FT):
                        msl = slice(m * P, (m + 1) * P)
                        ps2 = pp.tile([P, IBLK * K], F32, name=f"ps2_{m}", tag="ps")
                        for k2 in range(FT):
                            nc.tensor.matmul(
                                ps2, _mm(e2w[k2][:, msl], EDGE_DT),
                                _mm(h1[k2], EDGE_DT),
                                start=(k2 == 0), stop=(k2 == FT - 1),
                            )
                        h2m = cp.tile([P, IBLK * K], edge_store,
                                      name=f"h2_{m}", tag=f"h2_{m}")
                        nc.scalar.activation(h2m, ps2, AF.Relu, bias=bcol(4 + m))
                        h2.append(h2m)
                    for m in range(FT):
                        msl = slice(m * P, (m + 1) * P)
                        ps3 = pp.tile([P, IBLK * K], F32, name=f"ps3_{m}", tag="ps")
                        for k2 in range(FT):
                            nc.tensor.matmul(
                                ps3, _mm(e3w[k2][:, msl], EDGE_DT),
                                _mm(h2[k2], EDGE_DT),
                                start=(k2 == 0), stop=(k2 == FT - 1),
                            )
                        h3m = cp.tile([P, IBLK * K], edge_store,
                                      name=f"h3_{m}", tag=f"h3_{m}")
                        nc.scalar.activation(h3m, ps3, AF.Relu, bias=bcol(8 + m))
                        nc.vector.reduce_sum(
                            out=agg[m][:, i0 : i0 + IBLK],
                            in_=h3m[:].rearrange("p (i j) -> p i j", i=IBLK),
                            axis=mybir.AxisListType.X,
                        )

            # ---------------- LayerNorm over H (token stats via matmul) ----
            ones_col = pa.tile([P, 1], F32, name="ones_col", tag="ones_col")
            nc.vector.memset(ones_col, 1.0)
            ones_row = pa.tile([1, P], F32, name="ones_row", tag="ones_row")
            nc.vector.memset(ones_row, 1.0)

            sq = []
            for m in range(FT):
                sqm = pa.tile([P, TOK], F32, name=f"sq{m}", tag=f"sq{m}")
                nc.vector.tensor_mul(sqm, agg[m], agg[m])
                sq.append(sqm)
            ps_sum = pp.tile([1, TOK], F32, name="ps_sum", tag="ps")
            ps_ssq = pp.tile([1, TOK], F32, name="ps_ssq", tag="ps")
            for m in range(FT):
                nc.tensor.matmul(ps_sum, _mm(ones_col, SMALL_DT),
                                 _mm(agg[m], SMALL_DT),
                                 start=(m == 0), stop=(m == FT - 1))
            for m in range(FT):
                nc.tensor.matmul(ps_ssq, _mm(ones_col, SMALL_DT),
                                 _mm(sq[m], SMALL_DT),
                                 start=(m == 0), stop=(m == FT - 1))
            mean_r = pa.tile([1, TOK], F32, name="mean_r", tag="mean_r")
            nc.vector.tensor_scalar_mul(mean_r, ps_sum, 1.0 / H)
            msq_r = pa.tile([1, TOK], F32, name="msq_r", tag="msq_r")
            nc.vector.tensor_mul(msq_r, mean_r, mean_r)
            var_r = pa.tile([1, TOK], F32, name="var_r", tag="var_r")
            nc.vector.scalar_tensor_tensor(
                var_r, ps_ssq, 1.0 / H, msq_r, op0=ALU.mult, op1=ALU.subtract)
            eps_t = pa.tile([1, 1], F32, name="eps_t", tag="eps_t")
            nc.vector.memset(eps_t, EPS_S)
            sd_r = pa.tile([1, TOK], F32, name="sd_r", tag="sd_r")
            nc.scalar.activation(sd_r, var_r, AF.Sqrt, bias=eps_t)
            rstd_r = pa.tile([1, TOK], F32, name="rstd_r", tag="rstd_r")
            nc.vector.reciprocal(rstd_r, sd_r)

            ps_mb = pp.tile([P, TOK], F32, name="ps_mb", tag="ps")
            nc.tensor.matmul(ps_mb, _mm(ones_row, SMALL_DT),
                             _mm(mean_r, SMALL_DT), start=True, stop=True)
            mean_bc = pa.tile([P, TOK], F32, name="mean_bc", tag="mean_bc")
            nc.scalar.copy(mean_bc, ps_mb)
            ps_rb = pp.tile([P, TOK], F32, name="ps_rb", tag="ps")
            nc.tensor.matmul(ps_rb, _mm(ones_row, SMALL_DT),
                             _mm(rstd_r, SMALL_DT), start=True, stop=True)
            rstd_bc = pa.tile([P, TOK], F32, name="rstd_bc", tag="rstd_bc")
            nc.scalar.copy(rstd_bc, ps_rb)

            aggn = []
            for m in range(FT):
                anm = pa.tile([P, TOK], F32, name=f"aggn{m}", tag=f"aggn{m}")
                nc.vector.tensor_sub(anm, agg[m], mean_bc)
                nc.vector.tensor_mul(anm, anm, rstd_bc)
                aggn.append(anm)

            # ---------------- node MLP ----------------
            hn1 = []
            for m in range(FT):
                msl = slice(m * P, (m + 1) * P)
                psn = pp.tile([P, TOK], F32, name=f"psn1_{m}", tag="ps")
                for k2 in range(FT):
                    nc.tensor.matmul(psn, _mm(n1aw[k2][:, msl], SMALL_DT),
                                     _mm(aggn[k2], SMALL_DT),
                                     start=(k2 == 0), stop=False)
                nc.tensor.matmul(psn, _mm(n1ow[:, msl], SMALL_DT),
                                 _mm(o_all, SMALL_DT), start=False, stop=False)
                nc.tensor.matmul(psn, _mm(n1sw[:, msl], SMALL_DT),
                                 _mm(state_bc, SMALL_DT), start=False, stop=True)
                hm = pa.tile([P, TOK], F32, name=f"hn1_{m}", tag=f"hn1_{m}")
                nc.scalar.activation(hm, psn, AF.Relu, bias=bcol(12 + m))
                hn1.append(hm)

            pool_sum, pool_max = [], []
            for m2 in range(HH // P):
                msl = slice(m2 * P, (m2 + 1) * P)
                psn2 = pp.tile([P, TOK], F32, name=f"psn2_{m2}", tag="ps")
                for k2 in range(FT):
                    nc.tensor.matmul(psn2, _mm(n2w[k2][:, msl], SMALL_DT),
                                     _mm(hn1[k2], SMALL_DT),
                                     start=(k2 == 0), stop=(k2 == FT - 1))
                sm = pa.tile([P, BSH], F32, name=f"pool_s{m2}", tag=f"pool_s{m2}")
                nc.vector.reduce_sum(
                    out=sm, in_=psn2[:].rearrange("p (b k) -> p b k", b=BSH),
                    axis=mybir.AxisListType.X)
                mm_ = pa.tile([P, BSH], F32, name=f"pool_m{m2}", tag=f"pool_m{m2}")
                nc.vector.reduce_max(
                    out=mm_, in_=psn2[:].rearrange("p (b k) -> p b k", b=BSH),
                    axis=mybir.AxisListType.X)
                pool_sum.append(sm)
                pool_max.append(mm_)

            # ---------------- heads ----------------
            xs = []
            for m in range(FT):
                msl = slice(m * P, (m + 1) * P)
                pst = pp.tile([P, BSH], F32, name=f"pst{m}", tag="ps")
                nc.tensor.matmul(pst, _mm(layerw[:, msl], SMALL_DT),
                                 _mm(st_t, SMALL_DT), start=True, stop=True)
                xm = pa.tile([P, BSH], F32, name=f"xst{m}", tag=f"xst{m}")
                nc.scalar.activation(xm, pst, AF.Relu, bias=bcol(16 + m))
                xs.append(xm)
            xs += pool_sum + pool_max  # x = [st_feat, mean_pool, max_pool]

            def head(w1, w2, w3, bc1, bc2, tag):
                hl1 = []
                for m in range(2):
                    msl = slice(m * P, (m + 1) * P)
                    ph = pp.tile([P, BSH], F32, name=f"p{tag}1_{m}", tag="ps")
                    for k2 in range(2 * FT):
                        nc.tensor.matmul(ph, _mm(w1[k2][:, msl], SMALL_DT),
                                         _mm(xs[k2], SMALL_DT),
                                         start=(k2 == 0), stop=(k2 == 2 * FT - 1))
                    hm = pa.tile([P, BSH], F32, name=f"h{tag}1_{m}",
                                 tag=f"h{tag}1_{m}")
                    nc.scalar.activation(hm, ph, AF.Relu, bias=bcol(bc1 + m))
                    hl1.append(hm)
                ph2 = pp.tile([P, BSH], F32, name=f"p{tag}2", tag="ps")
                for k2 in range(2):
                    nc.tensor.matmul(ph2, _mm(w2[k2], SMALL_DT),
                                     _mm(hl1[k2], SMALL_DT),
                                     start=(k2 == 0), stop=(k2 == 1))
                hm2 = pa.tile([P, BSH], F32, name=f"h{tag}2", tag=f"h{tag}2")
                nc.scalar.activation(hm2, ph2, AF.Relu, bias=bcol(bc2))
                ph3 = pp.tile([NA, BSH], F32, name=f"p{tag}3", tag="ps")
                nc.tensor.matmul(ph3, _mm(w3, SMALL_DT), _mm(hm2, SMALL_DT),
                                 start=True, stop=True)
                return ph3

            ph3_mu = head(mu1w, mu2w, mu3w, 20, 22, "mu")
            mu_sb = pa.tile([NA, BSH], F32, name="mu_sb", tag="mu_sb")
            nc.scalar.activation(mu_sb, ph3_mu, AF.Identity, bias=bcol(23, rows=NA))
            nc.sync.dma_start(out=mu_d[:], in_=mu_sb)

            # softplus isn't co-loadable with sqrt in the ACT func sets;
            # emit the pre-softplus logits, host applies softplus+clip.
            ph3_s = head(s1w, s2w, s3w, 24, 26, "s")
            std_sb = pa.tile([NA, BSH], F32, name="std_sb", tag="std_sb")
            nc.scalar.activation(std_sb, ph3_s, AF.Identity, bias=bcol(27, rows=NA))
            nc.sync.dma_start(out=std_d[:], in_=std_sb)

    _split_excess_waits(nc)
    return nc


def prep_weights(inp):
    """Host-side weight preprocessing -> dict of replicated arrays."""
    f = lambda a: np.ascontiguousarray(a, dtype=np.float32)
    e1_w = np.asarray(inp["e1_w"], np.float32)
    n1_w = np.asarray(inp["n1_w"], np.float32)
    ln_g = np.asarray(inp["ln_g"], np.float32)
    ln_b = np.asarray(inp["ln_b"], np.float32)
    n2_b = np.asarray(inp["n2_b"], np.float32)
    mu1_w = np.asarray(inp["mu1_w"], np.float32)
    s1_w = np.asarray(inp["s1_w"], np.float32)

    d = {}
    d["e1AT"] = f(e1_w[:, :D].T)
    d["e1CT"] = f(e1_w[:, D:].T)
    edge_np = np.float32
    if EDGE_DT == "bf16":
        import ml_dtypes
        edge_np = ml_dtypes.bfloat16
    d["e2T"] = np.ascontiguousarray(
        np.asarray(inp["e2_w"], np.float32).T.reshape(FT, P, H), dtype=edge_np)
    d["e3T"] = np.ascontiguousarray(
        np.asarray(inp["e3_w"], np.float32).T.reshape(FT, P, H), dtype=edge_np)
    d["n1aT"] = f((n1_w[:, D : D + H] * ln_g[None, :]).T.reshape(FT, P, H))
    d["n1oT"] = f(n1_w[:, :D].T)
    d["n1sT"] = f(n1_w[:, D + H :].T)
    d["n2T"] = f(np.asarray(inp["n2_w"], np.float32).T.reshape(FT, P, HH))
    d["layerT"] = f(np.asarray(inp["layer_w"], np.float32).T)

    mu1 = mu1_w.copy()
    mu1[:, H : H + HH] *= 1.0 / K
    d["mu1T"] = f(mu1.T.reshape(2 * FT, P, 256))
    s1 = s1_w.copy()
    s1[:, H : H + HH] *= 1.0 / K
    d["s1T"] = f(s1.T.reshape(2 * FT, P, 256))
    d["mu2T"] = f(np.asarray(inp["mu2_w"], np.float32).T.reshape(2, P, 128))
    d["s2T"] = f(np.asarray(inp["s2_w"], np.float32).T.reshape(2, P, 128))
    d["mu3T"] = f(np.asarray(inp["mu3_w"], np.float32).T)
    d["s3T"] = f(np.asarray(inp["s3_w"], np.float32).T)

    n1_b_eff = np.asarray(inp["n1_b"], np.float32) + n1_w[:, D : D + H] @ ln_b
    mu1_b_eff = (np.asarray(inp["mu1_b"], np.float32)
                 + (mu1_w[:, H : H + HH] + mu1_w[:, H + HH :]) @ n2_b)
    s1_b_eff = (np.asarray(inp["s1_b"], np.float32)
                + (s1_w[:, H : H + HH] + s1_w[:, H + HH :]) @ n2_b)

    bp = np.zeros((P, 32), np.float32)
    bp[:, 0:4] = np.asarray(inp["e1_b"], np.float32).reshape(FT, P).T
    bp[:, 4:8] = np.asarray(inp["e2_b"], np.float32).reshape(FT, P).T
    bp[:, 8:12] = np.asarray(inp["e3_b"], np.float32).reshape(FT, P).T
    bp[:, 12:16] = n1_b_eff.reshape(FT, P).T
    bp[:, 16:20] = np.asarray(inp["layer_b"], np.float32).reshape(FT, P).T
    bp[:, 20:22] = mu1_b_eff.reshape(2, P).T
    bp[:, 22] = np.asarray(inp["mu2_b"], np.float32)
    bp[0:NA, 23] = np.asarray(inp["mu3_b"], np.float32)
    bp[:, 24:26] = s1_b_eff.reshape(2, P).T
    bp[:, 26] = np.asarray(inp["s2_b"], np.float32)
    bp[0:NA, 27] = np.asarray(inp["s3_b"], np.float32)
    d["bias_pack"] = bp
    return d


def make_in_maps(inputs):
    w = prep_weights(inputs)
    obs = np.ascontiguousarray(np.asarray(inputs["obs"], np.float32))
    state = np.asarray(inputs["state"], np.float32)
    in_maps = []
    for c in range(NCORES):
        m = dict(w)
        m["obs"] = np.ascontiguousarray(obs[c * BSH : (c + 1) * BSH])
        m["stateT"] = np.ascontiguousarray(state[c * BSH : (c + 1) * BSH].T)
        in_maps.append(m)
    return in_maps


_NC_CACHE = {}


def get_nc():
    key = (EDGE_DT, SMALL_DT)
    if key not in _NC_CACHE:
        _NC_CACHE[key] = build_bass()
    return _NC_CACHE[key]


def run(in_maps, trace=False, **kw):
    nc = get_nc()
    return run_bass_kernel_spmd(nc, in_maps, core_ids=list(range(NCORES)),
                                trace=trace, **kw)


def gather(res_list):
    mu = np.concatenate([r["mu"].T for r in res_list], axis=0)
    pre = np.concatenate([r["std"].T for r in res_list], axis=0).astype(np.float64)
    std = np.clip(np.log1p(np.exp(pre)) + 0.001, 0.1, 2.0)
    return mu.astype(np.float32), std.astype(np.float32)


def kernel(**inputs):
    res = run(make_in_maps(inputs))
    return gather(res.results)
